# revision 1
# baseline (speedup 1.0000x reference)
"""KQEnergyBlock Trainium2 Bass kernel, v2.

Math per batch element (see reference):
  Q = x Wq^T, K = x Wk^T            (N, D), heads = 64-col slices
  S_h = beta_h Q_h K_h^T ; A_h = softmax(S_h)
  T1 = AVc @ Wq   (AVc  = concat_h A_h K_h)
  T2 = ATQc @ Wk  (ATQc = concat_h A_h^T Q_h)
  out = T1 + T2 + relu(x Wm^T) Wm

Cost-model-driven design (TimelineSim): a matmul instruction costs only its
output free size, so every attention product keeps 128 output partitions and
transposes are offloaded to the DMA XBAR (dma_start(transpose=True)):
  - QT/KT via matmul; Qn/Kn derived from them by DMA transpose.
  - E = exp(beta*S) on ACT (scale=beta, accum_out=rowsum halves);
    A = E * (1/r) in place on DVE; A^T via one DMA transpose per head.
  - AVn[q,z] = sum_k A^T[k,q]^T Kn[k,z], ATQn[k,z] = sum_q A[q,k]^T Qn[q,z];
    head pairs accumulate into one [P,NC,2Z] psum tile, evicted with a single
    contiguous copy, then DMA-transposed into AVT/ATQT.
  - stage4: out = AVc@Wq + ATQc@Wk + hid@Wm accumulated in PSUM.

XBAR transpose dest constraints (probed): contiguous dest or mid-stride a
multiple of 256 bytes; in [128, F] -> out[po, mid, fo] = in[fo, mid*128+po].

Sharding: data-parallel over batch B=8, one element per core, no
collectives.
"""

import numpy as np
import ml_dtypes

import concourse.mybir as mybir
import concourse.tile as tile
from concourse import bacc
from concourse.bass_utils import run_bass_kernel_spmd

B, N, D = 8, 1024, 768
H, Z = 12, 64
HID = 3072
P = 128
DC = D // P     # 6
NC = N // P     # 8
BF = mybir.dt.bfloat16
F32 = mybir.dt.float32
Exp = mybir.ActivationFunctionType.Exp
Add = mybir.AluOpType.add

NPBF = ml_dtypes.bfloat16

_CACHE = {}


def _build(dbg=False):
    nc = bacc.Bacc("TRN2", target_bir_lowering=False, debug=False, num_devices=8)
    dbg_d = {}
    if dbg:
        for nm, shp in (("Qn", [P, NC, D]), ("Kn", [P, NC, D]),
                        ("ET0", [P, NC * NC, P]), ("E0", [P, NC, N]),
                        ("AVT", [P, DC, N]), ("ATQT", [P, DC, N]),
                        ("hid0", [P, N]), ("QT", [P, DC, N])):
            dbg_d[nm] = nc.dram_tensor("dbg_" + nm, shp, BF,
                                       kind="ExternalOutput")
    xT_d = nc.dram_tensor("xT", [D, N], BF, kind="ExternalInput")
    wqT_d = nc.dram_tensor("wqT", [D, D], BF, kind="ExternalInput")
    wkT_d = nc.dram_tensor("wkT", [D, D], BF, kind="ExternalInput")
    wq_d = nc.dram_tensor("wq", [D, D], BF, kind="ExternalInput")
    wk_d = nc.dram_tensor("wk", [D, D], BF, kind="ExternalInput")
    wmT_d = nc.dram_tensor("wmT", [D, HID], BF, kind="ExternalInput")
    wm_d = nc.dram_tensor("wm", [HID, D], BF, kind="ExternalInput")
    betav_d = nc.dram_tensor("betav", [P, H], F32, kind="ExternalInput")
    ident_d = nc.dram_tensor("ident", [P, P], BF, kind="ExternalInput")
    out_d = nc.dram_tensor("out", [N, D], BF, kind="ExternalOutput")

    xT_v = xT_d.ap().rearrange("(c p) n -> p c n", p=P)      # [128, 6, 1024]
    wqT_v = wqT_d.ap().rearrange("(c p) e -> p c e", p=P)
    wkT_v = wkT_d.ap().rearrange("(c p) e -> p c e", p=P)
    wq_v = wq_d.ap().rearrange("(c p) d -> p c d", p=P)
    wk_v = wk_d.ap().rearrange("(c p) d -> p c d", p=P)
    wmT_v = wmT_d.ap().rearrange("(c p) h -> p c h", p=P)    # [128, 6, 3072]
    wm_v = wm_d.ap().rearrange("(c p) d -> p c d", p=P)      # [128, 24, 768]
    out_v = out_d.ap().rearrange("(c p) d -> p c d", p=P)    # [128, 8, 768]

    with tile.TileContext(nc) as tc:
        with (
            tc.tile_pool(name="acts", bufs=1) as acts,
            tc.tile_pool(name="hd", bufs=1) as hd,
            tc.tile_pool(name="stream", bufs=3) as stream,
            tc.tile_pool(name="ps", bufs=2, space="PSUM") as ps,
            tc.tile_pool(name="dram", bufs=1, space="DRAM") as dram,
        ):
            # ---- persistent inputs (ordered so QT's first matmul can start
            # as soon as wqT + the first xT half arrive) ----
            xT = acts.tile([P, DC, N], BF)
            wqT = acts.tile([P, DC, D], BF)
            wkT = acts.tile([P, DC, D], BF)
            wq = acts.tile([P, DC, D], BF)
            wk = acts.tile([P, DC, D], BF)
            betav = acts.tile([P, H], F32)
            ident = acts.tile([P, P], BF)
            nc.sync.dma_start(wqT[:, :, 0:P], wqT_v[:, :, 0:P])
            nc.sync.dma_start(xT[:, :, 0:256], xT_v[:, :, 0:256])
            nc.sync.dma_start(wkT[:, :, 0:P], wkT_v[:, :, 0:P])
            nc.sync.dma_start(xT[:, :, 256:512], xT_v[:, :, 256:512])
            nc.sync.dma_start(xT[:, :, 512:N], xT_v[:, :, 512:N])
            nc.sync.dma_start(wqT[:, :, P:D], wqT_v[:, :, P:D])
            nc.sync.dma_start(wkT[:, :, P:D], wkT_v[:, :, P:D])
            nc.sync.dma_start(betav[:], betav_d.ap())
            nc.sync.dma_start(ident[:], ident_d.ap())
            nc.sync.dma_start(wq[:], wq_v)
            nc.sync.dma_start(wk[:], wk_v)

            QT = acts.tile([P, DC, N], BF)
            KT = acts.tile([P, DC, N], BF)
            Qn = acts.tile([P, NC, D], BF)
            Kn = acts.tile([P, NC, D], BF)
            # ET keeps its padded strided-3D XBAR dest (144 = 128+16)
            PT = P + 16
            # AVT/ATQT are assembled by PE transposes (identity matmuls)
            # into bf16-bitcast PSUM, evicted contiguously by DVE — this
            # keeps the saturated DMA device free for the A^T XBAR.
            AVTs = [acts.tile([P, NC, P], BF, name=f"AVT{c}")
                    for c in range(DC)]
            ATQTs = [acts.tile([P, NC, P], BF, name=f"ATQT{c}")
                     for c in range(DC)]
            hid_dram = dram.tile([2 * H, P, N], BF)

            def psE():
                return ps.tile([P, N], F32, tag="psE", name="pt", bufs=2)

            # ---- stage 1: projections (feature-major), naturals via XBAR ----
            # Only eo chunks 0,1 are computed up front (enough for heads
            # 0..3); the rest are emitted into the early head iterations'
            # PE bubbles via proj_pair().
            def proj_pair(eo, use_psav=False):
                for wT_sb, dstT, dstN in ((wqT, QT, Qn), (wkT, KT, Kn)):
                    if use_psav:
                        # the psav ring is idle until atq_part(0); using it
                        # decouples these fillers from the ACT-bound psE ring
                        pt = ps.tile([P, NC, 2 * Z], F32, tag="psav",
                                     name="pt", bufs=2)[:].rearrange(
                                         "p a b -> p (a b)")
                    else:
                        pt = psE()
                    for nh in range(2):
                        for do in range(DC):
                            nc.tensor.matmul(
                                pt[:, nh * 512:(nh + 1) * 512],
                                wT_sb[:, do, eo * P:(eo + 1) * P],
                                xT[:, do, nh * 512:(nh + 1) * 512],
                                start=(do == 0), stop=(do == DC - 1),
                            )
                    nc.vector.tensor_copy(dstT[:, eo, :], pt[:]
                                          if not use_psav else pt)
                    # natural layout via PE transposes (keeps the DMA device
                    # free in the congested early window)
                    if use_psav:
                        ptb = ps.tile([P, NC, 2 * Z], F32, tag="psav",
                                      name="ptb", bufs=2)[:].rearrange(
                                          "p a b -> p (a b)").bitcast(BF)
                    else:
                        ptb = psE()[:].bitcast(BF)
                    for qo in range(NC):
                        nc.tensor.transpose(
                            ptb[:, qo * P:(qo + 1) * P],
                            dstT[:, eo, qo * P:(qo + 1) * P], ident[:])
                    nc.vector.tensor_copy(
                        dstN[:, :, eo * P:(eo + 1) * P],
                        ptb[:, 0:NC * P].rearrange("p (a b) -> p a b", b=P))

            # eo=0 with tensor/nh interleave matched to input-load arrival
            # order (wqT0, xT_h0, wkT0, xT_h1): no PE queue-head blocking
            pts0 = {}
            for wT_sb, dstT, key in ((wqT, QT, "q"), (wkT, KT, "k")):
                pts0[key] = psE()
            # first QT quarter only needs the first xT quarter-load
    
            for seg0, seg1 in ((0, 256), (256, 512), (512, 1024)):
                for wT_sb, dstT, key in ((wqT, QT, "q"), (wkT, KT, "k")):
                    pt = pts0[key]
                    for do in range(DC):
                        nc.tensor.matmul(
                            pt[:, seg0:seg1],
                            wT_sb[:, do, 0:P],
                            xT[:, do, seg0:seg1],
                            start=(do == 0), stop=(do == DC - 1),
                        )
            for wT_sb, dstT, key in ((wqT, QT, "q"), (wkT, KT, "k")):
                dstN = Qn if key == "q" else Kn
                nc.vector.tensor_copy(dstT[:, 0, :], pts0[key][:])
                ptb = psE()[:].bitcast(BF)
                for qo in range(NC):
                    nc.tensor.transpose(
                        ptb[:, qo * P:(qo + 1) * P],
                        dstT[:, 0, qo * P:(qo + 1) * P], ident[:])
                nc.vector.tensor_copy(
                    dstN[:, :, 0:P],
                    ptb[:, 0:NC * P].rearrange("p (a b) -> p a b", b=P))

            # ---- stage 2+3: MLP layer 1 interleaved with per-head attention --
            # mlp1 chunk ho: hid rows [ho*128, (ho+1)*128) = relu(Wm x^T);
            # emitted as a list of closures so PE work can be interleaved
            # between S matmuls at fine grain.
            def mlp1_emit(ho):
                """Returns (steps, finish): steps = 12 matmul closures."""
                if ho % 2 == 0:
                    wt = stream.tile([P, DC, 2 * P], BF, tag="wmT", name="wt",
                                     bufs=2)
                    nc.sync.dma_start(wt[:], wmT_v[:, :, ho * P:(ho + 2) * P])
                    mlp1_emit.wt = wt
                wt = mlp1_emit.wt
                woff = (ho % 2) * P
                hchunk = stream.tile([P, N], BF, tag="hchunk", name="hchunk",
                                     bufs=2)
                phs = [None]
                steps = []
                for nh in range(2):
                    for do in range(DC):
                        def step(nh=nh, do=do):
                            if nh == 0 and do == 0:
                                phs[0] = psE()
                            nc.tensor.matmul(
                                phs[0][:, nh * 512:(nh + 1) * 512],
                                wt[:, do, woff:woff + P],
                                xT[:, do, nh * 512:(nh + 1) * 512],
                                start=(do == 0), stop=(do == DC - 1),
                            )
                        steps.append(step)

                def finish():
                    nc.vector.tensor_scalar_max(hchunk[:], phs[0][:], 0.0)
                    nc.sync.dma_start(hid_dram[ho], hchunk[:])
                return steps, finish

            def s_exp_norm(h, filler, ET_t):
                """E_h = exp(beta_h Q_h K_h^T), normalized in place per qo and
                DMA-transposed into ET_t in qo pairs as rows complete.
                `filler` yields PE closures (mlp1 matmuls) interleaved between
                S matmuls. Per-qo normalization needs a per-qo reciprocal of
                the accumulated rowsum."""
                zo = (h % 2) * Z
                c = h // 2
                QT_h = QT[zo:zo + Z, c, :]
                KT_h = KT[zo:zo + Z, c, :]
                E = hd.tile([P, NC, N], BF, tag="E", name="E", bufs=2)
                r_col = hd.tile([P, NC], F32, tag="r_col", name="r_col", bufs=2)
                rc_inv = hd.tile([P, NC], F32, tag="rc_inv", name="rc_inv",
                                 bufs=2)
                for qo in range(NC):
                    pt = psE()
                    for kh in range(2):
                        nc.tensor.matmul(
                            pt[:, kh * 512:(kh + 1) * 512],
                            QT_h[:, qo * P:(qo + 1) * P],
                            KT_h[:, kh * 512:(kh + 1) * 512],
                            start=True, stop=True,
                        )
                    nc.scalar.activation(
                        E[:, qo, :], pt[:], Exp,
                        scale=betav[:, h:h + 1],
                        accum_out=r_col[:, qo:qo + 1])
                    nc.vector.reciprocal(rc_inv[:, qo:qo + 1],
                                         r_col[:, qo:qo + 1])
                    nc.vector.tensor_scalar_mul(
                        E[:, qo, :], E[:, qo, :], rc_inv[:, qo:qo + 1])
                    for _ in range(3):
                        f = next(filler, None)
                        if f is not None:
                            f()
                return E, rc_inv

            def st_mm(h, ET_t, filler):
                """Odd heads: A^T computed on PE as exp(beta*S^T) UNNORMALIZED
                (row scale folded into the AV psum afterwards), written into
                ET_t with swapped (ko,qo) mid indexing."""
                zo = (h % 2) * Z
                c = h // 2
                QT_h = QT[zo:zo + Z, c, :]
                KT_h = KT[zo:zo + Z, c, :]
                for ko in range(NC):
                    pt = psE()
                    for qh in range(2):
                        nc.tensor.matmul(
                            pt[:, qh * 512:(qh + 1) * 512],
                            KT_h[:, ko * P:(ko + 1) * P],
                            QT_h[:, qh * 512:(qh + 1) * 512],
                            start=True, stop=True,
                        )
                    nc.scalar.activation(
                        ET_t[:, ko * NC:(ko + 1) * NC, 0:P],
                        pt[:].rearrange("p (a b) -> p a b", b=P), Exp,
                        scale=betav[:, h:h + 1])
                    f = next(filler, None)
                    if f is not None:
                        f()

            def atq_part(h, E):
                """ATQn for head h (depends only on E)."""
                off = (h % 2) * Z
                if h % 2 == 0:
                    atq_part.pav = ps.tile([P, NC, 2 * Z], F32, tag="psav",
                                           name="pav", bufs=2)
                    atq_part.patq = ps.tile([P, NC, 2 * Z], F32, tag="psav",
                                            name="patq", bufs=2)
                patq = atq_part.patq
                for ko in range(NC):
                    for qo in range(NC):
                        nc.tensor.matmul(
                            patq[:, ko, off:off + Z],
                            E[:, qo, ko * P:(ko + 1) * P],
                            Qn[:, qo, h * Z:(h + 1) * Z],
                            start=(qo == 0), stop=(qo == NC - 1),
                        )

            def av_part(h, ET_t, rc_inv):
                """AVn for head h; odd heads read the PE-computed unnormalized
                A^T (swapped indexing) and scale the psum rows afterwards; at
                odd h evict the pair and DMA-transpose into AVT/ATQT."""
                c = h // 2
                off = (h % 2) * Z
                odd = h % 2 == 1
                mm_path = False
                pav, patq = atq_part.pav, atq_part.patq
                for qo in range(NC):
                    for ko in range(NC):
                        idx = (ko * NC + qo) if mm_path else (qo * NC + ko)
                        nc.tensor.matmul(
                            pav[:, qo, off:off + Z],
                            ET_t[:, idx, 0:P],
                            Kn[:, ko, h * Z:(h + 1) * Z],
                            start=(ko == 0), stop=(ko == NC - 1),
                        )
                if mm_path:
                    for qo in range(NC):
                        nc.vector.tensor_scalar_mul(
                            pav[:, qo, off:off + Z], pav[:, qo, off:off + Z],
                            rc_inv[:, qo:qo + 1])
                if odd:
                    An = hd.tile([P, NC, 2 * Z], BF, tag="An", name="An",
                                 bufs=1)
                    Aq = hd.tile([P, NC, 2 * Z], BF, tag="Aq", name="Aq",
                                 bufs=1)
                    for tgt, src, psrc in ((ATQTs[c], Aq, patq),
                                           (AVTs[c], An, pav)):
                        ptb = psE()[:].bitcast(BF)
                        nc.vector.tensor_copy(src[:, 0:4, :], psrc[:, 0:4, :])
                        for qo in range(4):
                            nc.tensor.transpose(
                                ptb[:, qo * P:(qo + 1) * P], src[:, qo, :],
                                ident[:])
                        nc.vector.tensor_copy(src[:, 4:NC, :], psrc[:, 4:NC, :])
                        for qo in range(4, NC):
                            nc.tensor.transpose(
                                ptb[:, qo * P:(qo + 1) * P], src[:, qo, :],
                                ident[:])
                        nc.vector.tensor_copy(tgt[:], ptb[:, 0:NC * P])

            # software pipeline per iteration h:
            #   ATQn(h-1) [E-only] -> S/exp/norm(h) with mlp fillers ->
            #   AVn(h-1) [A^T had a full S-phase to transpose] -> evicts ->
            #   issue transpose of E(h).
            prev = None
            pre4 = []
            for h in range(H + 1):
                if prev is not None and h < 4:
                    atq_part(h - 1, prev[0])
                E = None
                filler = iter(())
                if h < H:
                    s0, f0 = mlp1_emit(2 * h)
                    s1, f1 = mlp1_emit(2 * h + 1)
                    filler = iter(s0 + [f0] + s1 + [f1])
                    E, rc_inv = s_exp_norm(h, filler, None)
                    for step in filler:
                        step()
                    if h == 0:
                        # remaining projection chunks fill the early-head
                        # bubble where PE would wait on the exp/norm chain;
                        # the psav ring keeps them off the ACT-bound psE ring
                        proj_pair(1, use_psav=True)
                        proj_pair(2, use_psav=True)
                        proj_pair(3, use_psav=True)
                    elif h in (1, 2):
                        proj_pair(h + 3)
                if prev is not None:
                    if h >= 4:
                        # late heads have no proj fillers left: the ET-
                        # independent ATQ work covers part of the A^T wait
                        atq_part(h - 1, prev[0])
                    av_part(h - 1, prev[1], prev[2])
                if h < H:
                    ET_t = hd.tile([P, NC * NC, PT], BF, tag="ET", name="ET",
                                   bufs=1)
                    nc.sync.dma_start(ET_t[:, :, 0:P], E[:], transpose=True)
                    prev = (E, ET_t, rc_inv)
                    if dbg and h == 0:
                        nc.sync.dma_start(dbg_d["E0"].ap(), E[:])
                        nc.sync.dma_start(dbg_d["ET0"].ap(), ET_t[:, :, 0:P])
                if h == H - 1:
                    # prefetch the first stage-4 streams before the last
                    # av_part so PE has data the moment stage 4 starts
                    for ho in range(4):
                        wmc = stream.tile([P, D], BF, tag="wmc", name="wmc",
                                          bufs=4)
                        nc.sync.dma_start(wmc[:], wm_v[:, ho, :])
                        hc = stream.tile([P, N], BF, tag="hc", name="hc",
                                         bufs=4)
                        nc.sync.dma_start(hc[:], hid_dram[ho])
                        pre4.append((wmc, hc))

            if dbg:
                nc.sync.dma_start(dbg_d["Qn"].ap(), Qn[:])
                nc.sync.dma_start(dbg_d["Kn"].ap(), Kn[:])
                nc.sync.dma_start(dbg_d["QT"].ap(), QT[:])
                avt_v = dbg_d["AVT"].ap()
                atqt_v = dbg_d["ATQT"].ap()
                for c in range(DC):
                    nc.sync.dma_start(avt_v[:, c, :].rearrange(
                        "p (m q) -> p m q", q=P), AVTs[c][:])
                    nc.sync.dma_start(atqt_v[:, c, :].rearrange(
                        "p (m q) -> p m q", q=P), ATQTs[c][:])
                hidc = stream.tile([P, N], BF, tag="hchunk", name="hdbg", bufs=2)
                nc.sync.dma_start(hidc[:], hid_dram[0])
                nc.sync.dma_start(dbg_d["hid0"].ap(), hidc[:])

            # ---- stage 4: out = AVc @ Wq + ATQc @ Wk + hid @ Wm ----
            # attn matmuls first (AVT/ATQT are ready before hid), hid stream
            # accumulates after; output DMAed straight from PSUM.
            for r4, nos in enumerate(([0, 1, 2, 3], [4, 5, 6, 7])):
                pouts = []
                for i in range(2):
                    t = psE()
                    pouts.append((t[:, 0:512], t[:, 512:768]))
                for i in range(2):
                    t = ps.tile([P, NC, 2 * Z], F32, tag="psav", name="po",
                                bufs=2)
                    pouts.append((t[:, 0:4, :], t[:, 4:6, :]))
                for i, no in enumerate(nos):
                    lo, hi = pouts[i]
                    for c2 in range(DC):
                        for lhss, w_sb in ((AVTs, wq), (ATQTs, wk)):
                            first = (c2 == 0 and lhss is AVTs)
                            nc.tensor.matmul(
                                lo,
                                lhss[c2][:, no, :],
                                w_sb[:, c2, 0:512],
                                start=first, stop=False,
                            )
                            nc.tensor.matmul(
                                hi,
                                lhss[c2][:, no, :],
                                w_sb[:, c2, 512:768],
                                start=first, stop=False,
                            )
                for ho in range(2 * H):
                    if r4 == 0 and ho < len(pre4):
                        wmc, hc = pre4[ho]
                    else:
                        wmc = stream.tile([P, D], BF, tag="wmc", name="wmc", bufs=4)
                        nc.sync.dma_start(wmc[:], wm_v[:, ho, :])
                        hc = stream.tile([P, N], BF, tag="hc", name="hc", bufs=4)
                        nc.sync.dma_start(hc[:], hid_dram[ho])
                    for i, no in enumerate(nos):
                        lo, hi = pouts[i]
                        last = (ho == 2 * H - 1)
                        nc.tensor.matmul(
                            lo, hc[:, no * P:(no + 1) * P], wmc[:, 0:512],
                            start=False, stop=last,
                        )
                        nc.tensor.matmul(
                            hi, hc[:, no * P:(no + 1) * P], wmc[:, 512:768],
                            start=False, stop=last,
                        )
                for g in range(2):
                    osb = stream.tile([P, 2, D], BF, tag="osb", name="osb",
                                      bufs=2)
                    for j in range(2):
                        i = 2 * g + j
                        lo, hi = pouts[i]
                        if i % 2 == 0:
                            nc.vector.tensor_copy(osb[:, j, 0:512], lo)
                            nc.vector.tensor_copy(osb[:, j, 512:768], hi)
                        else:
                            nc.scalar.copy(osb[:, j, 0:512], lo)
                            nc.scalar.copy(osb[:, j, 512:768], hi)
                    nc.sync.dma_start(
                        out_v[:, nos[2 * g]:nos[2 * g] + 2, :], osb[:])

    nc.compile()
    return nc


def _prep(x, Wq, Wk, betas, W_mlp):
    x = np.asarray(x, dtype=np.float32)
    Wq = np.asarray(Wq, dtype=np.float32)
    Wk = np.asarray(Wk, dtype=np.float32)
    betas = np.asarray(betas, dtype=np.float32)
    W_mlp = np.asarray(W_mlp, dtype=np.float32)

    wq = np.ascontiguousarray(Wq).astype(NPBF)
    wk = np.ascontiguousarray(Wk).astype(NPBF)
    wqT = np.ascontiguousarray(Wq.T).astype(NPBF)
    wkT = np.ascontiguousarray(Wk.T).astype(NPBF)
    wm = np.ascontiguousarray(W_mlp).astype(NPBF)
    wmT = np.ascontiguousarray(W_mlp.T).astype(NPBF)
    betav = np.ascontiguousarray(
        np.broadcast_to(betas[None, :], (P, H))).astype(np.float32)
    ident = np.eye(P, dtype=np.float32).astype(NPBF)

    in_maps = []
    for b in range(B):
        xT = np.ascontiguousarray(x[b].T).astype(NPBF)
        in_maps.append({
            "xT": xT, "wqT": wqT, "wkT": wkT, "wq": wq, "wk": wk,
            "wmT": wmT, "wm": wm, "betav": betav, "ident": ident,
        })
    return in_maps


def kernel(x, Wq, Wk, betas, W_mlp, _trace=False):
    if "nc" not in _CACHE:
        _CACHE["nc"] = _build()
    nc = _CACHE["nc"]
    in_maps = _prep(x, Wq, Wk, betas, W_mlp)
    res = run_bass_kernel_spmd(nc, in_maps, core_ids=list(range(B)), trace=_trace)
    out = np.stack([res.results[b]["out"] for b in range(B)], axis=0)
    _CACHE["last_result"] = res
    return out.astype(np.float32)



# revision 15
# speedup vs baseline: 1.1834x; 1.1834x over previous
"""KQEnergyBlock Trainium2 Bass kernel, v3 (fp8 DoubleRow redesign).

Math per batch element (see reference):
  Q = x Wq^T, K = x Wk^T            (N, D), heads = 64-col slices
  S_h = beta_h Q_h K_h^T ; A_h = softmax(S_h)
  T1 = AVc @ Wq   (AVc  = concat_h A_h K_h)
  T2 = ATQc @ Wk  (ATQc = concat_h A_h^T Q_h)
  out = T1 + T2 + relu(x Wm^T) Wm

Cost-model facts driving the design (TimelineSim):
  - matmul engine time = out_free_size * cycles_per_row; fp8e4 (e4m3) with
    perf_mode=DoubleRow costs 0.5 cycles/row while contracting [K,2,*]
    operand pairs -> 4x bf16 MAC throughput when pairs carry real data.
  - ACT exp costs ~1038ns per [128,1024] instruction: the 12 M softmax
    exps (~100us) are the floor; everything else overlaps around it.
  - XBAR DMA transpose costs 14ns per 16x128 tile and requires a 2-byte
    dtype: transposing fp8 PAIRS bitcast as uint16 halves the tile count
    AND lands A^T pre-paired in exactly the [K,2,M] layout DoubleRow wants.

Numerics (threshold 2e-2; measured ~1.3e-2 in simulation):
  - mlp is ~96% of output norm, so mlp1 runs in bf16 (kills x- and W1-
    quantization error, and bf16 needs no DR pairs so it is cheaper than a
    compensated fp8 path).
  - stage4 hid@Wm runs fp8-DR with a host-prepared residual-weight pass
    (wm8c = fp8(SW*C*Wm - wm8b)) accumulated into the same psum.
  - weights scaled by SW=32 (keeps everything under fp8e4m3's 240 max),
    softmax renorm constant C=16 folded into Qs / AV-evict / final evict.

Sharding: data-parallel over batch B=8, one element per core, no
collectives.
"""

import numpy as np
import ml_dtypes

import concourse.mybir as mybir
import concourse.tile as tile
from concourse import bacc
from concourse.bass_utils import run_bass_kernel_spmd

B, N, D = 8, 1024, 768
H, Z = 12, 64
HID = 3072
P = 128
DC = D // P      # 6
NC = N // P      # 8
HC = HID // P    # 24
BF = mybir.dt.bfloat16
F32 = mybir.dt.float32
F8 = mybir.dt.float8e4
U16 = mybir.dt.uint16
Exp = mybir.ActivationFunctionType.Exp
DR = mybir.MatmulPerfMode.DoubleRow
Mult = mybir.AluOpType.mult

NPBF = ml_dtypes.bfloat16
NPF8 = ml_dtypes.float8_e4m3

SW = 32.0        # weight scale into fp8
C = 16.0         # softmax renorm constant

_CACHE = {}


def _build(dbg=False):
    nc = bacc.Bacc("TRN2", target_bir_lowering=False, debug=False, num_devices=8)
    dbg_d = {}
    if dbg:
        for nm, shp, dt in (("QT8", [P, DC, 2, N], F8), ("KT8", [P, DC * N + 256], F8),
                            ("E80", [P, NC, N], F8), ("ET80", [P, 32, P], U16),
                            ("Qn8", [P, NC, D], F8), ("Knp", [P, DC, 4, P], U16),
                            ("hid8", [P, HC, N], F8), ("AVT8", [P, DC, NC, P], F8),
                            ("ATQT8", [P, DC, NC, P], F8), ("r0", [P, NC], F32)):
            dbg_d[nm] = nc.dram_tensor("dbg_" + nm, shp, dt, kind="ExternalOutput")

    xT8_d = nc.dram_tensor("xT8", [D, N], F8, kind="ExternalInput")
    xTb_d = nc.dram_tensor("xTb", [D, N], BF, kind="ExternalInput")
    wqT8_d = nc.dram_tensor("wqT8", [D, D], F8, kind="ExternalInput")
    wkT8_d = nc.dram_tensor("wkT8", [D, D], F8, kind="ExternalInput")
    wq8_d = nc.dram_tensor("wq8", [D, D], F8, kind="ExternalInput")
    wk8_d = nc.dram_tensor("wk8", [D, D], F8, kind="ExternalInput")
    wmTb_d = nc.dram_tensor("wmTb", [D, HID], BF, kind="ExternalInput")
    wm8b_d = nc.dram_tensor("wm8b", [HID, D], F8, kind="ExternalInput")
    wm8c_d = nc.dram_tensor("wm8c", [HID, D], F8, kind="ExternalInput")
    betap_d = nc.dram_tensor("betap", [P, H], F32, kind="ExternalInput")
    ident_d = nc.dram_tensor("ident", [P, P], BF, kind="ExternalInput")
    ident8_d = nc.dram_tensor("ident8", [P, P], F8, kind="ExternalInput")
    out_d = nc.dram_tensor("out", [N, D], BF, kind="ExternalOutput")

    xT8_v = xT8_d.ap().rearrange("(c p) n -> p c n", p=P)     # [128, 6, 1024]
    xTb_v = xTb_d.ap().rearrange("(c p) n -> p c n", p=P)
    wqT8_v = wqT8_d.ap().rearrange("(c p) e -> p c e", p=P)
    wkT8_v = wkT8_d.ap().rearrange("(c p) e -> p c e", p=P)
    wq8_v = wq8_d.ap().rearrange("(c p) d -> p c d", p=P)
    wk8_v = wk8_d.ap().rearrange("(c p) d -> p c d", p=P)
    wmTb_v = wmTb_d.ap().rearrange("(c p) h -> p c h", p=P)   # [128, 6, 3072]
    wm8b_v = wm8b_d.ap().rearrange("(c p) d -> p c d", p=P)   # [128, 24, 768]
    wm8c_v = wm8c_d.ap().rearrange("(c p) d -> p c d", p=P)
    out_v = out_d.ap().rearrange("(c p) d -> p c d", p=P)     # [128, 8, 768]

    with tile.TileContext(nc) as tc:
        with (
            tc.tile_pool(name="acts", bufs=1) as acts,
            tc.tile_pool(name="hd", bufs=1) as hd,
            tc.tile_pool(name="stream", bufs=1) as stream,
            tc.tile_pool(name="ps", bufs=1, space="PSUM") as ps,
        ):
            # ---- persistent tiles ----
            xT8 = acts.tile([P, DC, N], F8)
            xTb = acts.tile([P, DC, N], BF)
            wqT8 = acts.tile([P, DC, D], F8)
            wkT8 = acts.tile([P, DC, D], F8)
            wq8 = acts.tile([P, DC, D], F8)
            wk8 = acts.tile([P, DC, D], F8)
            wm8b = acts.tile([P, HC, D], F8)
            wm8c = acts.tile([P, HC, D], F8)
            betap = acts.tile([P, H], F32)
            ident = acts.tile([P, P], BF)
            ident8 = acts.tile([P, P], F8)
            ones8 = acts.tile([P, 2, 16], F8)
            # projections: QT8 slot1 is a zero pad (DoubleRow partner row);
            # KT8 gets a 256-col zero tail so the k-window rhs can overrun.
            QT8 = acts.tile([P, DC, 2, N], F8)
            KT8 = acts.tile([P, DC * N + 256], F8)
            Qn8 = acts.tile([P, NC, D], F8)
            Knp = acts.tile([P, DC, 4, P], U16)     # paired Kn from XBAR
            hid8 = acts.tile([P, HC, N], F8)
            # Qs: persistent [q, qo, 128]; head h writes z-cols
            # [(h%2)*64, +64) so consecutive heads use disjoint halves and
            # the ATQ stationary can span all 128 columns (dual-fp8 matmuls
            # must write psum starting at partition 0).
            Qs8 = acts.tile([P, NC, P], F8)
            AVT8 = acts.tile([P, DC, NC, P], F8)
            ATQT8 = acts.tile([P, DC, NC, P], F8)

            def psE():
                return ps.tile([P, N], F32, tag="psE", name="pt", bufs=2)

            def ps_pav():
                return ps.tile([P, NC, Z], F32, tag="pav", name="pav", bufs=2)

            def ps_atq():
                # [:, 0:256] = ATQT kseg accumulator; [:, 256:264] doubles as
                # the rowsum accumulator (separate allocations, same ring).
                return ps.tile([P, 264], F32, tag="patq", name="patq", bufs=2)

            # ---- input loads (ordered for proj critical path) ----
            nc.sync.dma_start(wqT8[:], wqT8_v)
            nc.sync.dma_start(xT8[:], xT8_v)
            nc.sync.dma_start(wkT8[:], wkT8_v)
            nc.sync.dma_start(betap[:], betap_d.ap())
            nc.sync.dma_start(ident[:], ident_d.ap())
            nc.sync.dma_start(ident8[:], ident8_d.ap())
            nc.sync.dma_start(xTb[:], xTb_v)
            nc.gpsimd.memset(QT8[:, :, 1, :], 0.0)
            nc.gpsimd.memset(KT8[:, DC * N:], 0.0)
            nc.vector.memset(ones8[:], 1.0)
            nc.gpsimd.memset(Qs8[:, :, Z:], 0.0)

            KT8v = KT8[:]  # [128, 6400]

            # ---- projections: QT/KT feature-major, fp8 DoubleRow ----
            for eo in range(DC):
                for wT, isq in ((wqT8, True), (wkT8, False)):
                    pt = psE()
                    # segment-major: a region's full start->stop accumulation
                    # chain must complete before the next start in the same
                    # 2KB psum zero-region
                    for ns in range(4):
                        for cp in range(DC // 2):
                            nc.tensor.matmul(
                                pt[:, ns * 256:(ns + 1) * 256],
                                wT[:, 2 * cp:2 * cp + 2, eo * P:(eo + 1) * P],
                                xT8[:, 2 * cp:2 * cp + 2, ns * 256:(ns + 1) * 256],
                                start=(cp == 0), stop=(cp == DC // 2 - 1),
                                perf_mode=DR,
                            )
                    if isq:
                        nc.vector.tensor_copy(QT8[:, eo, 0, :], pt[:])
                    else:
                        nc.vector.tensor_copy(
                            KT8v[:, eo * N:(eo + 1) * N], pt[:])

            # Qn: natural-layout Q via PE transposes (fp8). FP8 transpose
            # writes with element step 2; evict reads the even-byte lanes.
            for eo in range(DC):
                ptb = psE()[:].bitcast(F8)   # [128, 4096]
                ptv = ptb[:, 0:NC * 2 * P].rearrange(
                    "p (a f s) -> p a s f", a=NC, s=2)   # [128, 8, 2, 128]
                for qo in range(NC):
                    nc.tensor.transpose(
                        ptv[:, qo, 0, :],
                        QT8[:, eo, 0, qo * P:(qo + 1) * P], ident8[:])
                nc.vector.tensor_copy(
                    Qn8[:, :, eo * P:(eo + 1) * P], ptv[:, :, 0, :])

            # Knp: k-paired natural K via XBAR on u16-bitcast KT8 chunks.
            KT8u = KT8[:].bitcast(U16)       # [128, 3200]
            for c in range(DC):
                nc.sync.dma_start(
                    Knp[:, c, :, :],
                    KT8u[:, c * 512:(c + 1) * 512], transpose=True)

            # ---- mlp1 (bf16): hid8[ho] = relu(Wm x^T) chunks, as fillers ----
            def mlp1_emit(ho):
                wt = stream.tile([P, DC, P], BF, tag="wmT", name="wt", bufs=4)
                nc.sync.dma_start(wt[:], wmTb_v[:, :, ho * P:(ho + 1) * P])
                phs = [None]
                steps = []
                for nh in range(2):
                    for do in range(DC):
                        def step(nh=nh, do=do):
                            if nh == 0 and do == 0:
                                phs[0] = psE()
                            nc.tensor.matmul(
                                phs[0][:, nh * 512:(nh + 1) * 512],
                                wt[:, do, :],
                                xTb[:, do, nh * 512:(nh + 1) * 512],
                                start=(do == 0), stop=(do == DC - 1),
                            )
                        steps.append(step)

                def finish(ho=ho):
                    nc.vector.tensor_scalar_max(hid8[:, ho, :], phs[0][:], 0.0)
                return steps, finish

            # ---- per-head attention pieces ----
            def s_exp(h, filler):
                """E8 = exp(beta/SW^2 * S) in fp8, S via zero-padded DR."""
                zo = (h % 2) * Z
                c = h // 2
                E8 = hd.tile([P, NC, N], F8, tag="E8", name="E8", bufs=2)
                for qo in range(NC):
                    pt = psE()
                    for j in range(4):
                        nc.tensor.matmul(
                            pt[:, j * 256:(j + 1) * 256],
                            QT8[zo:zo + Z, c, :, qo * P:(qo + 1) * P],
                            KT8v[zo:zo + Z, c * N + j * 256:c * N + j * 256 + 512]
                            .rearrange("p (a b) -> p a b", a=2),
                            start=True, stop=True, perf_mode=DR,
                        )
                    nc.scalar.activation(
                        E8[:, qo, :], pt[:], Exp, scale=betap[:, h:h + 1])
                    for _ in range(3):
                        f = next(filler, None)
                        if f is not None:
                            f()
                return E8

            def et_xbar(h, E8):
                ET8 = hd.tile([P, 32, P], U16, tag="ET8", name="ET8", bufs=2)
                nc.sync.dma_start(
                    ET8[:], E8[:].bitcast(U16).rearrange("p a b -> p (a b)"),
                    transpose=True)
                return ET8

            def et_lhs(ET8, qo, kcp, s):
                # A^T stationary: [128 kp, 2 (kc-pair, 256B stride), 128 q
                # (step 2, parity s)] — dual-fp8 LdWeights needs the pair
                # stride 16B-aligned, so the byte-interleave is consumed as
                # two parity lanes instead.
                sl = ET8[:].bitcast(F8)[:, qo * 4 + 2 * kcp:qo * 4 + 2 * kcp + 2, :]
                return sl.rearrange("p a (f s) -> p a s f", s=2)[:, :, s, :]

            def rowsum(h, ET8):
                """r[q] per qo via tiny PE ones-matmuls on packed A^T."""
                pr = ps_atq()[:, 256:256 + NC]
                for qo in range(NC):
                    for i, (kcp, s) in enumerate(
                            ((0, 0), (0, 1), (1, 0), (1, 1))):
                        nc.tensor.matmul(
                            pr[:, qo:qo + 1], et_lhs(ET8, qo, kcp, s),
                            ones8[:, :, 0:1],
                            start=(i == 0), stop=(i == 3), perf_mode=DR,
                        )
                return pr

            def make_qs(h, pr):
                """rc = C/r ; Qs[z-half of head h] = Qn * rc (fp8)."""
                rc = hd.tile([P, NC], F32, tag="rc", name="rc", bufs=2)
                zo = (h % 2) * Z
                nc.vector.reciprocal(rc[:], pr[:])
                for qo in range(NC):
                    nc.vector.tensor_scalar(
                        Qs8[:, qo, zo:zo + Z], Qn8[:, qo, h * Z:(h + 1) * Z],
                        rc[:, qo:qo + 1], C, Mult, Mult)
                return rc

            def atq_direct(h, E8):
                """ATQT[z,k] = sum_q Qs[q,z] E8[q,k], evicted per kseg.
                The stationary spans all 128 Qs columns; the other head's
                64 rows compute junk that the evict skips."""
                zo = (h % 2) * Z
                c = h // 2
                for ks in range(4):
                    pt = ps_atq()[:, 0:256]
                    for qp in range(4):
                        nc.tensor.matmul(
                            pt[:],
                            Qs8[:, 2 * qp:2 * qp + 2, :],
                            E8[:, 2 * qp:2 * qp + 2, ks * 256:(ks + 1) * 256],
                            start=(qp == 0), stop=(qp == 3), perf_mode=DR,
                        )
                    nc.vector.tensor_copy(
                        ATQT8[zo:zo + Z, c, 2 * ks:2 * ks + 2, :],
                        pt[zo:zo + Z, :].rearrange("p (a b) -> p a b", b=P))

            def av_part(h, ET8):
                """AV[q,z] = sum_k A^T[k,q] K[k,z] via packed ET8 + Knp."""
                c = h // 2
                if h % 2 == 0:
                    av_part.pav_e = ps_pav()
                    pav = av_part.pav_e
                else:
                    av_part.pav_o = ps_pav()
                    pav = av_part.pav_o
                Knpf = Knp[:].bitcast(F8)   # [128, 6, 4, 256]
                for qo in range(NC):
                    for i, (kcp, s) in enumerate(
                            ((0, 0), (0, 1), (1, 0), (1, 1))):
                        rhs = Knpf[:, c, 2 * kcp:2 * kcp + 2, :].rearrange(
                            "p a (f s) -> p a s f", s=2)[
                                :, :, s, (h % 2) * Z:(h % 2) * Z + Z]
                        nc.tensor.matmul(
                            pav[:, qo, :], et_lhs(ET8, qo, kcp, s), rhs,
                            start=(i == 0), stop=(i == 3), perf_mode=DR,
                        )

            def av_evict(hpair, rc_e, rc_o):
                """An = pav * (C/r) per head, PE-transpose into AVT8."""
                c = hpair
                An8 = hd.tile([P, NC, 2 * Z], F8, tag="An8", name="An8", bufs=2)
                for qo in range(NC):
                    nc.vector.tensor_scalar(
                        An8[:, qo, 0:Z], av_part.pav_e[:, qo, :],
                        rc_e[:, qo:qo + 1], C, Mult, Mult)
                    nc.vector.tensor_scalar(
                        An8[:, qo, Z:2 * Z], av_part.pav_o[:, qo, :],
                        rc_o[:, qo:qo + 1], C, Mult, Mult)
                ptb = psE()[:].bitcast(F8)
                ptv = ptb[:, 0:NC * 2 * P].rearrange(
                    "p (a f s) -> p a s f", a=NC, s=2)   # [128, 8, 2, 128]
                for qo in range(NC):
                    nc.tensor.transpose(
                        ptv[:, qo, 0, :], An8[:, qo, :], ident8[:])
                nc.vector.tensor_copy(AVT8[:, c, :, :], ptv[:, :, 0, :])

            # ---- software pipeline over heads ----
            # phase h: S(h)+exp(h) with mlp1 fillers; then rowsum/Qs/ATQ/AV
            # for h-1 (its XBAR landed during exp(h)); pair-evict at odd h-1.
            mlp_chunks = [mlp1_emit(ho) for ho in range(2)]
            prev = None           # (E8, ET8) of h-1
            rcs = {}
            next_ho = 2
            for h in range(H + 1):
                filler_items = []
                if h < H:
                    # two mlp1 chunks per phase (prefetch weight 2 ahead)
                    for _ in range(2):
                        if mlp_chunks:
                            s, f = mlp_chunks.pop(0)
                            filler_items.extend(s)
                            filler_items.append(f)
                        if next_ho < HC:
                            mlp_chunks.append(mlp1_emit(next_ho))
                            next_ho += 1
                    filler = iter(filler_items)
                    E8 = s_exp(h, filler)
                    for f in filler:
                        f()
                if prev is not None:
                    hp = h - 1
                    pE8, pET8 = prev
                    pr = rowsum(hp, pET8)
                    rcs[hp] = make_qs(hp, pr)
                    atq_direct(hp, pE8)
                    av_part(hp, pET8)
                    if hp % 2 == 1:
                        av_evict(hp // 2, rcs[hp - 1], rcs[hp])
                        if dbg and hp == 1:
                            nc.sync.dma_start(dbg_d["r0"].ap(), rcs[0][:])
                if h < H:
                    ET8 = et_xbar(h, E8)
                    prev = (E8, ET8)
                    if dbg and h == 0:
                        nc.sync.dma_start(dbg_d["E80"].ap(), E8[:])
                        nc.sync.dma_start(dbg_d["ET80"].ap(), ET8[:])
                if h == 2:
                    # stage4 weights land during the early phases
                    nc.sync.dma_start(wq8[:], wq8_v)
                    nc.sync.dma_start(wk8[:], wk8_v)
                if h == 4:
                    nc.sync.dma_start(wm8b[:], wm8b_v)
                if h == 6:
                    nc.sync.dma_start(wm8c[:], wm8c_v)

            if dbg:
                nc.sync.dma_start(dbg_d["QT8"].ap(), QT8[:])
                nc.sync.dma_start(dbg_d["KT8"].ap(), KT8[:])
                nc.sync.dma_start(dbg_d["Qn8"].ap(), Qn8[:])
                nc.sync.dma_start(dbg_d["Knp"].ap(), Knp[:])
                nc.sync.dma_start(dbg_d["hid8"].ap(), hid8[:])
                nc.sync.dma_start(dbg_d["AVT8"].ap(), AVT8[:])
                nc.sync.dma_start(dbg_d["ATQT8"].ap(), ATQT8[:])

            # ---- stage 4: out = (AVc@Wq + ATQc@Wk + hid@(Wm*C)) / (SW^2 C)
            OSC = 1.0 / (SW * SW * C)
            for rnd in range(4):
                pouts = [psE() for _ in range(2)]
                for i, po in enumerate(pouts):
                    no = 2 * rnd + i
                    pt = po[:, 0:D]
                    for ds in range(3):
                        dsl = slice(ds * 256, (ds + 1) * 256)
                        for cp in range(DC // 2):
                            for lhsT, w in ((AVT8, wq8), (ATQT8, wk8)):
                                nc.tensor.matmul(
                                    pt[:, dsl],
                                    lhsT[:, 2 * cp:2 * cp + 2, no, :],
                                    w[:, 2 * cp:2 * cp + 2, dsl],
                                    start=(cp == 0 and lhsT is AVT8),
                                    stop=False, perf_mode=DR,
                                )
                        for wm_t in (wm8b, wm8c):
                            for hp in range(HC // 2):
                                nc.tensor.matmul(
                                    pt[:, dsl],
                                    hid8[:, 2 * hp:2 * hp + 2, no * P:(no + 1) * P],
                                    wm_t[:, 2 * hp:2 * hp + 2, dsl],
                                    start=False,
                                    stop=(wm_t is wm8c and hp == HC // 2 - 1),
                                    perf_mode=DR,
                                )
                osb = stream.tile([P, 2, D], BF, tag="osb", name="osb", bufs=2)
                nc.vector.tensor_scalar_mul(osb[:, 0, :], pouts[0][:, 0:D], OSC)
                nc.scalar.activation(
                    osb[:, 1, :], pouts[1][:, 0:D],
                    mybir.ActivationFunctionType.Copy, scale=OSC)
                nc.sync.dma_start(out_v[:, 2 * rnd:2 * rnd + 2, :], osb[:])

    nc.compile()
    return nc


def _prep(x, Wq, Wk, betas, W_mlp):
    x = np.asarray(x, dtype=np.float32)
    Wq = np.asarray(Wq, dtype=np.float32)
    Wk = np.asarray(Wk, dtype=np.float32)
    betas = np.asarray(betas, dtype=np.float32)
    W_mlp = np.asarray(W_mlp, dtype=np.float32)

    wqT8 = np.ascontiguousarray(Wq.T * SW).astype(NPF8)
    wkT8 = np.ascontiguousarray(Wk.T * SW).astype(NPF8)
    wq8 = np.ascontiguousarray(Wq * SW).astype(NPF8)
    wk8 = np.ascontiguousarray(Wk * SW).astype(NPF8)
    wmTb = np.ascontiguousarray(W_mlp.T * SW).astype(NPBF)
    wm8b = np.ascontiguousarray(W_mlp * (SW * C)).astype(NPF8)
    wm8c = np.ascontiguousarray(
        W_mlp * (SW * C) - wm8b.astype(np.float32)).astype(NPF8)
    betap = np.ascontiguousarray(np.broadcast_to(
        (betas / (SW * SW))[None, :], (P, H))).astype(np.float32)
    ident = np.eye(P, dtype=np.float32).astype(NPBF)
    ident8 = np.eye(P, dtype=np.float32).astype(NPF8)

    in_maps = []
    for b in range(B):
        xT = np.ascontiguousarray(x[b].T)
        in_maps.append({
            "xT8": xT.astype(NPF8), "xTb": xT.astype(NPBF),
            "wqT8": wqT8, "wkT8": wkT8, "wq8": wq8, "wk8": wk8,
            "wmTb": wmTb, "wm8b": wm8b, "wm8c": wm8c,
            "betap": betap, "ident": ident, "ident8": ident8,
        })
    return in_maps


def kernel(x, Wq, Wk, betas, W_mlp, _trace=False, _dbg=False):
    key = "nc_dbg" if _dbg else "nc"
    if key not in _CACHE:
        _CACHE[key] = _build(dbg=_dbg)
    nc = _CACHE[key]
    in_maps = _prep(x, Wq, Wk, betas, W_mlp)
    core_ids = list(range(B)) if not _dbg else [0]
    res = run_bass_kernel_spmd(nc, in_maps[:len(core_ids)], core_ids=core_ids,
                               trace=_trace)
    _CACHE["last_result"] = res
    if _dbg:
        return res
    out = np.stack([res.results[b]["out"] for b in range(B)], axis=0)
    return out.astype(np.float32)


# revision 18
# speedup vs baseline: 1.3526x; 1.1429x over previous
"""KQEnergyBlock Trainium2 Bass kernel, v3 (fp8 DoubleRow redesign).

Math per batch element (see reference):
  Q = x Wq^T, K = x Wk^T            (N, D), heads = 64-col slices
  S_h = beta_h Q_h K_h^T ; A_h = softmax(S_h)
  T1 = AVc @ Wq   (AVc  = concat_h A_h K_h)
  T2 = ATQc @ Wk  (ATQc = concat_h A_h^T Q_h)
  out = T1 + T2 + relu(x Wm^T) Wm

Cost-model facts driving the design (TimelineSim):
  - matmul engine time = out_free_size * cycles_per_row; fp8e4 (e4m3) with
    perf_mode=DoubleRow costs 0.5 cycles/row while contracting [K,2,*]
    operand pairs -> 4x bf16 MAC throughput when pairs carry real data.
  - ACT exp costs ~1038ns per [128,1024] instruction: the 12 M softmax
    exps (~100us) are the floor; everything else overlaps around it.
  - XBAR DMA transpose costs 14ns per 16x128 tile and requires a 2-byte
    dtype: transposing fp8 PAIRS bitcast as uint16 halves the tile count
    AND lands A^T pre-paired in exactly the [K,2,M] layout DoubleRow wants.

Numerics (threshold 2e-2; measured ~1.3e-2 in simulation):
  - mlp is ~96% of output norm, so mlp1 runs in bf16 (kills x- and W1-
    quantization error, and bf16 needs no DR pairs so it is cheaper than a
    compensated fp8 path).
  - stage4 hid@Wm runs fp8-DR with a host-prepared residual-weight pass
    (wm8c = fp8(SW*C*Wm - wm8b)) accumulated into the same psum.
  - weights scaled by SW=32 (keeps everything under fp8e4m3's 240 max),
    softmax renorm constant C=16 folded into Qs / AV-evict / final evict.

Sharding: data-parallel over batch B=8, one element per core, no
collectives.
"""

import numpy as np
import ml_dtypes

import concourse.mybir as mybir
import concourse.tile as tile
from concourse import bacc
from concourse.bass_utils import run_bass_kernel_spmd

B, N, D = 8, 1024, 768
H, Z = 12, 64
HID = 3072
P = 128
DC = D // P      # 6
NC = N // P      # 8
HC = HID // P    # 24
BF = mybir.dt.bfloat16
F32 = mybir.dt.float32
F8 = mybir.dt.float8e4
U16 = mybir.dt.uint16
Exp = mybir.ActivationFunctionType.Exp
DR = mybir.MatmulPerfMode.DoubleRow
Mult = mybir.AluOpType.mult

NPBF = ml_dtypes.bfloat16
NPF8 = ml_dtypes.float8_e4m3

SW = 32.0        # weight scale into fp8
C = 16.0         # softmax renorm constant

_CACHE = {}


def _build(dbg=False):
    nc = bacc.Bacc("TRN2", target_bir_lowering=False, debug=False, num_devices=8)
    dbg_d = {}
    if dbg:
        for nm, shp, dt in (("QT8", [P, DC, 2, N], F8), ("KT8", [P, DC * N + 256], F8),
                            ("E80", [P, NC, N], F8), ("ET80", [P, 32, P], U16),
                            ("Qn8", [P, NC, D], F8), ("Knp", [P, DC, 4, P], U16),
                            ("hid8", [P, HC, N], F8), ("AVT8", [P, DC, NC, P], F8),
                            ("ATQT8", [P, DC, NC, P], F8), ("r0", [P, NC], F32)):
            dbg_d[nm] = nc.dram_tensor("dbg_" + nm, shp, dt, kind="ExternalOutput")

    xT8_d = nc.dram_tensor("xT8", [D, N], F8, kind="ExternalInput")
    xTb_d = nc.dram_tensor("xTb", [D, N], BF, kind="ExternalInput")
    wqT8_d = nc.dram_tensor("wqT8", [D, D], F8, kind="ExternalInput")
    wkT8_d = nc.dram_tensor("wkT8", [D, D], F8, kind="ExternalInput")
    wq8_d = nc.dram_tensor("wq8", [D, D], F8, kind="ExternalInput")
    wk8_d = nc.dram_tensor("wk8", [D, D], F8, kind="ExternalInput")
    wmTb_d = nc.dram_tensor("wmTb", [D, HID], BF, kind="ExternalInput")
    wm8b_d = nc.dram_tensor("wm8b", [HID, D], F8, kind="ExternalInput")
    wm8c_d = nc.dram_tensor("wm8c", [HID, D], F8, kind="ExternalInput")
    betap_d = nc.dram_tensor("betap", [P, H], F32, kind="ExternalInput")
    ident_d = nc.dram_tensor("ident", [P, P], BF, kind="ExternalInput")
    ident8_d = nc.dram_tensor("ident8", [P, P], F8, kind="ExternalInput")
    out_d = nc.dram_tensor("out", [N, D], BF, kind="ExternalOutput")

    xT8_v = xT8_d.ap().rearrange("(c p) n -> p c n", p=P)     # [128, 6, 1024]
    xTb_v = xTb_d.ap().rearrange("(c p) n -> p c n", p=P)
    wqT8_v = wqT8_d.ap().rearrange("(c p) e -> p c e", p=P)
    wkT8_v = wkT8_d.ap().rearrange("(c p) e -> p c e", p=P)
    wq8_v = wq8_d.ap().rearrange("(c p) d -> p c d", p=P)
    wk8_v = wk8_d.ap().rearrange("(c p) d -> p c d", p=P)
    wmTb_v = wmTb_d.ap().rearrange("(c p) h -> p c h", p=P)   # [128, 6, 3072]
    wm8b_v = wm8b_d.ap().rearrange("(c p) d -> p c d", p=P)   # [128, 24, 768]
    wm8c_v = wm8c_d.ap().rearrange("(c p) d -> p c d", p=P)
    out_v = out_d.ap().rearrange("(c p) d -> p c d", p=P)     # [128, 8, 768]

    with tile.TileContext(nc) as tc:
        with (
            tc.tile_pool(name="acts", bufs=1) as acts,
            tc.tile_pool(name="hd", bufs=1) as hd,
            tc.tile_pool(name="stream", bufs=1) as stream,
            tc.tile_pool(name="ps", bufs=1, space="PSUM") as ps,
        ):
            # ---- persistent tiles ----
            xT8 = acts.tile([P, DC, N], F8)
            xTb = acts.tile([P, DC, N], BF)
            wqT8 = acts.tile([P, DC, D], F8)
            wkT8 = acts.tile([P, DC, D], F8)
            wq8 = acts.tile([P, DC, D], F8)
            wk8 = acts.tile([P, DC, D], F8)
            wm8b = acts.tile([P, HC, D], F8)
            wm8c = acts.tile([P, HC, D], F8)
            betap = acts.tile([P, H], F32)
            ident = acts.tile([P, P], BF)
            ident8 = acts.tile([P, P], F8)
            ones8 = acts.tile([P, 2, 16], F8)
            # projections: QT8 slot1 is a zero pad (DoubleRow partner row);
            # KT8 gets a 256-col zero tail so the k-window rhs can overrun.
            QT8 = acts.tile([P, DC, 2, N], F8)
            KT8 = acts.tile([P, DC * N + 256], F8)
            Qn8 = acts.tile([P, NC, D], F8)
            Knp = acts.tile([P, DC, 4, P], U16)     # paired Kn from XBAR
            hid8 = acts.tile([P, HC, N], F8)
            # Qs: persistent [q, qo, 128]; head h writes z-cols
            # [(h%2)*64, +64) so consecutive heads use disjoint halves and
            # the ATQ stationary can span all 128 columns (dual-fp8 matmuls
            # must write psum starting at partition 0).
            Qs8 = acts.tile([P, NC, P], F8)
            AVT8 = acts.tile([P, DC, NC, P], F8)
            ATQT8 = acts.tile([P, DC, NC, P], F8)

            def psE():
                return ps.tile([P, N], F32, tag="psE", name="pt", bufs=2)

            def ps_pav():
                return ps.tile([P, NC, Z], F32, tag="pav", name="pav", bufs=2)

            def ps_atq():
                # [:, 0:256] = ATQT kseg accumulator; [:, 256:264] doubles as
                # the rowsum accumulator (separate allocations, same ring).
                return ps.tile([P, 264], F32, tag="patq", name="patq", bufs=2)

            # ---- input loads (ordered for proj critical path) ----
            nc.sync.dma_start(wqT8[:], wqT8_v)
            nc.sync.dma_start(xT8[:], xT8_v)
            nc.sync.dma_start(wkT8[:], wkT8_v)
            nc.sync.dma_start(betap[:], betap_d.ap())
            nc.sync.dma_start(ident[:], ident_d.ap())
            nc.sync.dma_start(ident8[:], ident8_d.ap())
            nc.sync.dma_start(xTb[:], xTb_v)
            nc.gpsimd.memset(QT8[:, :, 1, :], 0.0)
            nc.gpsimd.memset(KT8[:, DC * N:], 0.0)
            nc.vector.memset(ones8[:], 1.0)
            nc.gpsimd.memset(Qs8[:, :, Z:], 0.0)

            KT8v = KT8[:]  # [128, 6400]

            # ---- projections: QT/KT feature-major, fp8 DoubleRow ----
            for eo in range(DC):
                for wT, isq in ((wqT8, True), (wkT8, False)):
                    pt = psE()
                    # segment-major: a region's full start->stop accumulation
                    # chain must complete before the next start in the same
                    # 2KB psum zero-region
                    for ns in range(4):
                        for cp in range(DC // 2):
                            nc.tensor.matmul(
                                pt[:, ns * 256:(ns + 1) * 256],
                                wT[:, 2 * cp:2 * cp + 2, eo * P:(eo + 1) * P],
                                xT8[:, 2 * cp:2 * cp + 2, ns * 256:(ns + 1) * 256],
                                start=(cp == 0), stop=(cp == DC // 2 - 1),
                                perf_mode=DR,
                            )
                    if isq:
                        nc.vector.tensor_copy(QT8[:, eo, 0, :], pt[:])
                    else:
                        nc.vector.tensor_copy(
                            KT8v[:, eo * N:(eo + 1) * N], pt[:])

            # Qn: natural-layout Q via PE transposes (fp8). FP8 transpose
            # writes with element step 2; evict reads the even-byte lanes.
            for eo in range(DC):
                ptb = psE()[:].bitcast(F8)   # [128, 4096]
                ptv = ptb[:, 0:NC * 2 * P].rearrange(
                    "p (a f s) -> p a s f", a=NC, s=2)   # [128, 8, 2, 128]
                for qo in range(NC):
                    nc.tensor.transpose(
                        ptv[:, qo, 0, :],
                        QT8[:, eo, 0, qo * P:(qo + 1) * P], ident8[:])
                nc.vector.tensor_copy(
                    Qn8[:, :, eo * P:(eo + 1) * P], ptv[:, :, 0, :])

            # Knp: k-paired natural K via XBAR on u16-bitcast KT8 chunks.
            KT8u = KT8[:].bitcast(U16)       # [128, 3200]
            for c in range(DC):
                nc.sync.dma_start(
                    Knp[:, c, :, :],
                    KT8u[:, c * 512:(c + 1) * 512], transpose=True)

            # ---- mlp1 (bf16): hid8[ho] = relu(Wm x^T), emitted as four
            # 256-wide chains per ho on the patq psum ring so the psE ring
            # stays a clean S<->exp ping-pong (PE queues are in-order; a
            # long-lived mlp psum slot there serializes exp).
            def mlp1_emit(ho):
                wt = stream.tile([P, DC, P], BF, tag="wmT", name="wt", bufs=4)
                nc.sync.dma_start(wt[:], wmTb_v[:, :, ho * P:(ho + 1) * P])

                def chain(ns, ho=ho, wt=wt):
                    pt = ps_atq()[:, 0:256]
                    for do in range(DC):
                        nc.tensor.matmul(
                            pt[:], wt[:, do, :],
                            xTb[:, do, ns * 256:(ns + 1) * 256],
                            start=(do == 0), stop=(do == DC - 1),
                        )
                    nc.vector.tensor_scalar_max(
                        hid8[:, ho, ns * 256:(ns + 1) * 256], pt[:], 0.0)
                return [lambda ns=ns: chain(ns) for ns in range(4)]

            # ---- per-head attention pieces ----
            def s_exp(h, filler):
                """E8 = exp(beta/SW^2 * S) in fp8, S via zero-padded DR."""
                zo = (h % 2) * Z
                c = h // 2
                E8 = hd.tile([P, NC, N], F8, tag="E8", name="E8", bufs=2)
                for qo in range(NC):
                    pt = psE()
                    for j in range(4):
                        nc.tensor.matmul(
                            pt[:, j * 256:(j + 1) * 256],
                            QT8[zo:zo + Z, c, :, qo * P:(qo + 1) * P],
                            KT8v[zo:zo + Z, c * N + j * 256:c * N + j * 256 + 512]
                            .rearrange("p (a b) -> p a b", a=2),
                            start=True, stop=True, perf_mode=DR,
                        )
                    nc.scalar.activation(
                        E8[:, qo, :], pt[:], Exp, scale=betap[:, h:h + 1])
                    f = next(filler, None)
                    if f is not None:
                        f()
                return E8

            def et_xbar(h, E8):
                ET8 = hd.tile([P, 32, P], U16, tag="ET8", name="ET8", bufs=2)
                nc.sync.dma_start(
                    ET8[:], E8[:].bitcast(U16).rearrange("p a b -> p (a b)"),
                    transpose=True)
                return ET8

            def et_lhs(ET8, qo, kcp, s):
                # A^T stationary: [128 kp, 2 (kc-pair, 256B stride), 128 q
                # (step 2, parity s)] — dual-fp8 LdWeights needs the pair
                # stride 16B-aligned, so the byte-interleave is consumed as
                # two parity lanes instead.
                sl = ET8[:].bitcast(F8)[:, qo * 4 + 2 * kcp:qo * 4 + 2 * kcp + 2, :]
                return sl.rearrange("p a (f s) -> p a s f", s=2)[:, :, s, :]

            def rowsum(h, ET8):
                """r[q] per qo via tiny PE ones-matmuls on packed A^T."""
                pr = ps_atq()[:, 256:256 + NC]
                for qo in range(NC):
                    for i, (kcp, s) in enumerate(
                            ((0, 0), (0, 1), (1, 0), (1, 1))):
                        nc.tensor.matmul(
                            pr[:, qo:qo + 1], et_lhs(ET8, qo, kcp, s),
                            ones8[:, :, 0:1],
                            start=(i == 0), stop=(i == 3), perf_mode=DR,
                        )
                return pr

            def make_qs(h, pr):
                """rc = C/r ; Qs[z-half of head h] = Qn * rc (fp8)."""
                rc = hd.tile([P, NC], F32, tag="rc", name="rc", bufs=2)
                zo = (h % 2) * Z
                nc.vector.reciprocal(rc[:], pr[:])
                for qo in range(NC):
                    nc.vector.tensor_scalar(
                        Qs8[:, qo, zo:zo + Z], Qn8[:, qo, h * Z:(h + 1) * Z],
                        rc[:, qo:qo + 1], C, Mult, Mult)
                return rc

            def atq_direct(h, E8):
                """ATQT[z,k] = sum_q Qs[q,z] E8[q,k], evicted per kseg.
                The stationary spans all 128 Qs columns; the other head's
                64 rows compute junk that the evict skips."""
                zo = (h % 2) * Z
                c = h // 2
                for ks in range(4):
                    pt = ps_atq()[:, 0:256]
                    for qp in range(4):
                        nc.tensor.matmul(
                            pt[:],
                            Qs8[:, 2 * qp:2 * qp + 2, :],
                            E8[:, 2 * qp:2 * qp + 2, ks * 256:(ks + 1) * 256],
                            start=(qp == 0), stop=(qp == 3), perf_mode=DR,
                        )
                    nc.vector.tensor_copy(
                        ATQT8[zo:zo + Z, c, 2 * ks:2 * ks + 2, :],
                        pt[zo:zo + Z, :].rearrange("p (a b) -> p a b", b=P))

            def av_part(h, ET8):
                """AV[q,z] = sum_k A^T[k,q] K[k,z] via packed ET8 + Knp."""
                c = h // 2
                if h % 2 == 0:
                    av_part.pav_e = ps_pav()
                    pav = av_part.pav_e
                else:
                    av_part.pav_o = ps_pav()
                    pav = av_part.pav_o
                Knpf = Knp[:].bitcast(F8)   # [128, 6, 4, 256]
                for qo in range(NC):
                    for i, (kcp, s) in enumerate(
                            ((0, 0), (0, 1), (1, 0), (1, 1))):
                        rhs = Knpf[:, c, 2 * kcp:2 * kcp + 2, :].rearrange(
                            "p a (f s) -> p a s f", s=2)[
                                :, :, s, (h % 2) * Z:(h % 2) * Z + Z]
                        nc.tensor.matmul(
                            pav[:, qo, :], et_lhs(ET8, qo, kcp, s), rhs,
                            start=(i == 0), stop=(i == 3), perf_mode=DR,
                        )

            def av_evict(hpair, rc_e, rc_o):
                """An = pav * (C/r) per head, PE-transpose into AVT8."""
                c = hpair
                An8 = hd.tile([P, NC, 2 * Z], F8, tag="An8", name="An8", bufs=2)
                for qo in range(NC):
                    nc.vector.tensor_scalar(
                        An8[:, qo, 0:Z], av_part.pav_e[:, qo, :],
                        rc_e[:, qo:qo + 1], C, Mult, Mult)
                    nc.vector.tensor_scalar(
                        An8[:, qo, Z:2 * Z], av_part.pav_o[:, qo, :],
                        rc_o[:, qo:qo + 1], C, Mult, Mult)
                ptb = psE()[:].bitcast(F8)
                ptv = ptb[:, 0:NC * 2 * P].rearrange(
                    "p (a f s) -> p a s f", a=NC, s=2)   # [128, 8, 2, 128]
                for qo in range(NC):
                    nc.tensor.transpose(
                        ptv[:, qo, 0, :], An8[:, qo, :], ident8[:])
                nc.vector.tensor_copy(AVT8[:, c, :, :], ptv[:, :, 0, :])

            # ---- software pipeline over heads ----
            # phase h: S(h)+exp(h) with mlp1 fillers; then rowsum/Qs/ATQ/AV
            # for h-1 (its XBAR landed during exp(h)); pair-evict at odd h-1.
            mlp_chunks = [mlp1_emit(ho) for ho in range(2)]
            prev = None           # (E8, ET8) of h-1
            rcs = {}
            next_ho = 2
            for h in range(H + 1):
                filler_items = []
                if h < H:
                    # two mlp1 chunks (8 chains) per phase: one chain per qo
                    for _ in range(2):
                        if mlp_chunks:
                            filler_items.extend(mlp_chunks.pop(0))
                        if next_ho < HC:
                            mlp_chunks.append(mlp1_emit(next_ho))
                            next_ho += 1
                    filler = iter(filler_items)
                    E8 = s_exp(h, filler)
                    for f in filler:
                        f()
                if prev is not None:
                    hp = h - 1
                    pE8, pET8 = prev
                    pr = rowsum(hp, pET8)
                    rcs[hp] = make_qs(hp, pr)
                    atq_direct(hp, pE8)
                    av_part(hp, pET8)
                    if hp % 2 == 1:
                        av_evict(hp // 2, rcs[hp - 1], rcs[hp])
                        if dbg and hp == 1:
                            nc.sync.dma_start(dbg_d["r0"].ap(), rcs[0][:])
                if h < H:
                    ET8 = et_xbar(h, E8)
                    prev = (E8, ET8)
                    if dbg and h == 0:
                        nc.sync.dma_start(dbg_d["E80"].ap(), E8[:])
                        nc.sync.dma_start(dbg_d["ET80"].ap(), ET8[:])
                if h == 2:
                    # stage4 weights land during the early phases
                    nc.sync.dma_start(wq8[:], wq8_v)
                    nc.sync.dma_start(wk8[:], wk8_v)
                if h == 4:
                    nc.sync.dma_start(wm8b[:], wm8b_v)
                if h == 6:
                    nc.sync.dma_start(wm8c[:], wm8c_v)

            if dbg:
                nc.sync.dma_start(dbg_d["QT8"].ap(), QT8[:])
                nc.sync.dma_start(dbg_d["KT8"].ap(), KT8[:])
                nc.sync.dma_start(dbg_d["Qn8"].ap(), Qn8[:])
                nc.sync.dma_start(dbg_d["Knp"].ap(), Knp[:])
                nc.sync.dma_start(dbg_d["hid8"].ap(), hid8[:])
                nc.sync.dma_start(dbg_d["AVT8"].ap(), AVT8[:])
                nc.sync.dma_start(dbg_d["ATQT8"].ap(), ATQT8[:])

            # ---- stage 4: out = (AVc@Wq + ATQc@Wk + hid@(Wm*C)) / (SW^2 C)
            OSC = 1.0 / (SW * SW * C)
            for rnd in range(4):
                pouts = [psE() for _ in range(2)]
                for i, po in enumerate(pouts):
                    no = 2 * rnd + i
                    pt = po[:, 0:D]
                    for ds in range(3):
                        dsl = slice(ds * 256, (ds + 1) * 256)
                        for cp in range(DC // 2):
                            for lhsT, w in ((AVT8, wq8), (ATQT8, wk8)):
                                nc.tensor.matmul(
                                    pt[:, dsl],
                                    lhsT[:, 2 * cp:2 * cp + 2, no, :],
                                    w[:, 2 * cp:2 * cp + 2, dsl],
                                    start=(cp == 0 and lhsT is AVT8),
                                    stop=False, perf_mode=DR,
                                )
                        for wm_t in (wm8b, wm8c):
                            for hp in range(HC // 2):
                                nc.tensor.matmul(
                                    pt[:, dsl],
                                    hid8[:, 2 * hp:2 * hp + 2, no * P:(no + 1) * P],
                                    wm_t[:, 2 * hp:2 * hp + 2, dsl],
                                    start=False,
                                    stop=(wm_t is wm8c and hp == HC // 2 - 1),
                                    perf_mode=DR,
                                )
                osb = stream.tile([P, 2, D], BF, tag="osb", name="osb", bufs=2)
                nc.vector.tensor_scalar_mul(osb[:, 0, :], pouts[0][:, 0:D], OSC)
                nc.scalar.activation(
                    osb[:, 1, :], pouts[1][:, 0:D],
                    mybir.ActivationFunctionType.Copy, scale=OSC)
                nc.sync.dma_start(out_v[:, 2 * rnd:2 * rnd + 2, :], osb[:])

    nc.compile()
    return nc


def _prep(x, Wq, Wk, betas, W_mlp):
    x = np.asarray(x, dtype=np.float32)
    Wq = np.asarray(Wq, dtype=np.float32)
    Wk = np.asarray(Wk, dtype=np.float32)
    betas = np.asarray(betas, dtype=np.float32)
    W_mlp = np.asarray(W_mlp, dtype=np.float32)

    wqT8 = np.ascontiguousarray(Wq.T * SW).astype(NPF8)
    wkT8 = np.ascontiguousarray(Wk.T * SW).astype(NPF8)
    wq8 = np.ascontiguousarray(Wq * SW).astype(NPF8)
    wk8 = np.ascontiguousarray(Wk * SW).astype(NPF8)
    wmTb = np.ascontiguousarray(W_mlp.T * SW).astype(NPBF)
    wm8b = np.ascontiguousarray(W_mlp * (SW * C)).astype(NPF8)
    wm8c = np.ascontiguousarray(
        W_mlp * (SW * C) - wm8b.astype(np.float32)).astype(NPF8)
    betap = np.ascontiguousarray(np.broadcast_to(
        (betas / (SW * SW))[None, :], (P, H))).astype(np.float32)
    ident = np.eye(P, dtype=np.float32).astype(NPBF)
    ident8 = np.eye(P, dtype=np.float32).astype(NPF8)

    in_maps = []
    for b in range(B):
        xT = np.ascontiguousarray(x[b].T)
        in_maps.append({
            "xT8": xT.astype(NPF8), "xTb": xT.astype(NPBF),
            "wqT8": wqT8, "wkT8": wkT8, "wq8": wq8, "wk8": wk8,
            "wmTb": wmTb, "wm8b": wm8b, "wm8c": wm8c,
            "betap": betap, "ident": ident, "ident8": ident8,
        })
    return in_maps


def kernel(x, Wq, Wk, betas, W_mlp, _trace=False, _dbg=False):
    key = "nc_dbg" if _dbg else "nc"
    if key not in _CACHE:
        _CACHE[key] = _build(dbg=_dbg)
    nc = _CACHE[key]
    in_maps = _prep(x, Wq, Wk, betas, W_mlp)
    core_ids = list(range(B)) if not _dbg else [0]
    res = run_bass_kernel_spmd(nc, in_maps[:len(core_ids)], core_ids=core_ids,
                               trace=_trace)
    _CACHE["last_result"] = res
    if _dbg:
        return res
    out = np.stack([res.results[b]["out"] for b in range(B)], axis=0)
    return out.astype(np.float32)
